# revision 13
# baseline (speedup 1.0000x reference)
"""Trainium2 SPMD kernel: modulated conv via 1D-Winograd F(2,3) on width.

Same factorization as kernel.py (style modulation folded into x, shared
weight, sigma demodulation at the output), but the 3-wide conv dimension is
Winograd-transformed so the PE does 12 taps of 512-wide work per (cin,cout)
pair instead of 18 half-width taps: 192 conv matmuls instead of 288.

Per output row pair (width tiles tx of 2):
    out[2tx]   = w0*x[2tx]  + w1*x[2tx+1] + w2*x[2tx+2]
    out[2tx+1] = w0*x[2tx+1]+ w1*x[2tx+2] + w2*x[2tx+3]
with xs = x*style*RC (zero-padded), the F(2,3) decomposition (input scales
folded into the x transform so the weight transform is integer-only):
    T0 = xs[2tx] - xs[2tx+2]         T1 = (xs[2tx+1] + xs[2tx+2])/2
    T2 = (xs[2tx+2] - xs[2tx+1])/2   T3 = xs[2tx+1] - xs[2tx+3]
    U0 = w0   U1 = w0+w1+w2   U2 = w0-w1+w2   U3 = w2     (per dy row)
    M[p] = sum_{c,dy} U[p,dy]^T @ T_p[dy:dy+32]           (PE, N=512)
    Y0 = M1+M2+M0   Y1 = M1-M2-M3
    out[:, y, 0::2] = sigma*Y0,  out[:, y, 1::2] = sigma*Y1

16 psum accumulation groups (4 p x 4 cout-chunks) run in three bank waves
(7+7+2) on 7 banks; wave 1 = {M1,M2} so the cross-wave partials are plain
add/sub. Sigma = 8th bank, computed as in kernel.py.
"""

from contextlib import ExitStack

import ml_dtypes
import numpy as np

import concourse.bass as bass
import concourse.tile as tile
from concourse import bacc, mybir
from concourse.bass_utils import run_bass_kernel_spmd

B = 8
CIN = 512
COUT = 512
KK = 3
H = 32
W = 32
PIX = H * W
NCH = 4
TAPS = KK * KK
NTX = W // 2  # 16 width tiles
RC = float(1.0 / np.sqrt(CIN * KK * KK))
EPS = 1e-8
F32 = mybir.dt.float32
BF16 = mybir.dt.bfloat16
AF = mybir.ActivationFunctionType
ALU = mybir.AluOpType

TRACE = False
LAST_RESULTS = None


def _body(ctx, tc, x_d, st_d, wt_d, out_d):
    nc = tc.nc
    const = ctx.enter_context(tc.tile_pool(name="const", bufs=1))
    wpool = ctx.enter_context(tc.tile_pool(name="wpool", bufs=1))
    upool = ctx.enter_context(tc.tile_pool(name="upool", bufs=1))
    xpool = ctx.enter_context(tc.tile_pool(name="xpool", bufs=1))
    sqpool = ctx.enter_context(tc.tile_pool(name="sqpool", bufs=2))
    ypool = ctx.enter_context(tc.tile_pool(name="ypool", bufs=1))
    opool = ctx.enter_context(tc.tile_pool(name="opool", bufs=2))
    psum = ctx.enter_context(
        tc.tile_pool(name="psum", bufs=1, space=bass.MemorySpace.PSUM)
    )
    sigpsum = ctx.enter_context(
        tc.tile_pool(name="sigpsum", bufs=1, space=bass.MemorySpace.PSUM)
    )

    # --- PE pre-warm ---
    warm = const.tile([1, 129], BF16, tag="warm")
    with tc.high_priority():
        nc.vector.memset(warm[:], 1.0)
    ones_sig = const.tile([128, 1], BF16, tag="ones_sig")
    nc.vector.memset(ones_sig[:], 1.0)
    # sig_q even columns 0,2,4,6 hold the sigma reduction (8-byte PSUM
    # cacheline alignment for the N=1 matmul outputs); columns 8:136 are
    # scratch
    # for the warmup matmuls (PSUM pool tiles are bank-granular).
    sig_q = sigpsum.tile([128, 136], F32, tag="sigq")
    for _ in range(14):
        nc.tensor.matmul(
            sig_q[0:1, 8:136], warm[:, 0:1], warm[:, 1:129], start=True, stop=True
        )

    # --- style scales ---
    st = const.tile([128, NCH], F32, tag="st")
    st_rc = const.tile([128, NCH], F32, tag="st_rc")
    with tc.high_priority(offset=2):
        nc.sync.dma_start(st[:], st_d[:])
        nc.vector.tensor_scalar_mul(st_rc[:], st[:], RC)
    st2 = const.tile([128, NCH], BF16, tag="st2")
    nc.vector.tensor_mul(st2[:], st[:], st[:])

    # --- input DMAs ---
    wt = [
        wpool.tile([128, TAPS, COUT], BF16, tag=f"wt{c}", name=f"wt{c}")
        for c in range(NCH)
    ]
    for lo, hi in [(0, 3), (3, 6), (6, 9)]:
        nc.scalar.dma_start(wt[0][:, lo:hi], wt_d[:, 0, lo:hi])

    xs, xh, xst = [], [], []
    for c in range(NCH):
        xc = xpool.tile([128, H + 2, W + 2], BF16, tag=f"xs{c}", name=f"xs{c}")
        eng = nc.vector if c == 0 else nc.gpsimd
        with tc.high_priority(offset=3):
            eng.memset(xc[:], 0.0)
        xs.append(xc)
        xh.append(xpool.tile([128, H + 2, W + 2], BF16, tag=f"xh{c}", name=f"xh{c}"))
        xst.append(xpool.tile([128, H, W], BF16, tag=f"xst{c}", name=f"xst{c}"))

    with tc.high_priority(offset=3):
        nc.sync.dma_start(xst[0][:], x_d[0].rearrange("p (h w) -> p h w", h=H))
    for c in range(1, NCH):
        nc.scalar.dma_start(xst[c][:], x_d[c].rearrange("p (h w) -> p h w", h=H))
        nc.scalar.dma_start(wt[c][:], wt_d[:, c])

    # --- x-side transforms (DVE). even/odd are stride-2 column views of the
    # padded picture; all four T_p are [128, 34, 16]. T0/T3 (raw-weight
    # wave 1) come first so the PE can start early; xh/T1/T2 follow.
    T = [[None] * 4 for _ in range(NCH)]
    for c in range(NCH):
        for p in range(4):
            T[c][p] = xpool.tile(
                [128, H + 2, NTX], BF16, tag=f"T{p}_{c}", name=f"T{p}_{c}"
            )
    for c in range(NCH):
        with tc.high_priority(offset=3):
            nc.vector.tensor_scalar_mul(
                xs[c][:, 1 : H + 1, 1 : W + 1], xst[c][:], st_rc[:, c : c + 1]
            )
            ev = xs[c][:].rearrange("p r (t k) -> p r t k", k=2)[:, :, :, 0]
            od = xs[c][:].rearrange("p r (t k) -> p r t k", k=2)[:, :, :, 1]
            nc.vector.tensor_sub(T[c][0][:], ev[:, :, 0:NTX], ev[:, :, 1 : NTX + 1])
            nc.vector.tensor_sub(T[c][3][:], od[:, :, 0:NTX], od[:, :, 1 : NTX + 1])
    # wave-2 inputs, interleaved per chunk so U[c] / T1/T2[c] / sigma
    # partials land in arrival order: xh/T1/T2 (DVE), batched integer weight
    # transform (one [128,3,512] strided add chain per chunk; U0/U3 are
    # views of wt), and the sigma square/tap-sum pipeline.
    U1a, U2a = {}, {}
    w2s = {}
    for c in range(NCH):
        nc.vector.tensor_scalar_mul(xh[c][:], xs[c][:], 0.5)
        evh = xh[c][:].rearrange("p r (t k) -> p r t k", k=2)[:, :, :, 0]
        odh = xh[c][:].rearrange("p r (t k) -> p r t k", k=2)[:, :, :, 1]
        nc.vector.tensor_add(T[c][1][:], odh[:, :, 0:NTX], evh[:, :, 1 : NTX + 1])
        nc.vector.tensor_sub(T[c][2][:], evh[:, :, 1 : NTX + 1], odh[:, :, 0:NTX])
        a = sqpool.tile([128, 3, COUT], BF16, tag="ua", name=f"ua{c}")
        u1 = upool.tile([128, 3, COUT], BF16, tag=f"u1_{c}", name=f"u1_{c}")
        u2 = upool.tile([128, 3, COUT], BF16, tag=f"u2_{c}", name=f"u2_{c}")
        wv = wt[c][:].rearrange("p (dy dx) o -> p dy dx o", dx=3)
        nc.vector.tensor_add(a[:], wv[:, :, 0], wv[:, :, 2])
        nc.vector.tensor_add(u1[:], a[:], wv[:, :, 1])
        nc.vector.tensor_sub(u2[:], a[:], wv[:, :, 1])
        U1a[c] = u1
        U2a[c] = u2
        # sigma partials for this chunk: ACT squares, DVE tap-sums
        w2st = sqpool.tile([128, TAPS, COUT], BF16, tag="w2st", name=f"w2st{c}")
        nc.scalar.activation(w2st[:], wt[c][:], AF.Square)
        acc3 = sqpool.tile([128, 3, COUT], BF16, tag="acc3", name=f"acc3{c}")
        nc.vector.tensor_add(acc3[:], w2st[:, 0:3], w2st[:, 3:6])
        nc.vector.tensor_add(acc3[:], acc3[:], w2st[:, 6:9])
        acc = sqpool.tile([128, COUT], BF16, tag=f"w2s{c}", name=f"w2s{c}")
        nc.vector.tensor_add(acc[:], acc3[:, 0], acc3[:, 1])
        nc.vector.tensor_add(acc[:], acc[:], acc3[:, 2])
        w2s[c] = acc

    def lhsT(p, c, dy, oc):
        ocs = slice(oc * 128, (oc + 1) * 128)
        if p == 0:
            return wt[c][:, 3 * dy + 0, ocs]
        if p == 3:
            return wt[c][:, 3 * dy + 2, ocs]
        return (U1a[c] if p == 1 else U2a[c])[:, dy, ocs]

    def sig_col(oc):
        for c in range(NCH):
            nc.tensor.matmul(
                sig_q[:, 2 * oc : 2 * oc + 1],
                w2s[c][:, oc * 128 : (oc + 1) * 128],
                st2[:, c : c + 1],
                start=(c == 0),
                stop=(c == NCH - 1),
            )

    sig_f = const.tile([128, NCH], F32, tag="sig_f")
    sig_s = const.tile([128, NCH], F32, tag="sig_s")
    sig_t = const.tile([128, NCH], F32, tag="sig_t")

    # --- conv waves. 16 groups (p, oc). Wave 1 = {M0, M3} (raw-weight lhsT
    # views, so no dependency on the GpSimd weight transform), wave 2 =
    # {M3 oc3, M1, M2 oc0..2}, wave 3 = {M1, M2 oc3}.
    WAVE1 = [(0, 0), (0, 1), (0, 2), (0, 3), (3, 0), (3, 1), (3, 2)]
    WAVE2 = [(3, 3), (1, 0), (1, 1), (1, 2), (2, 0), (2, 1), (2, 2)]
    WAVE3 = [(1, 3), (2, 3)]
    TAGOF = {}
    for i, g in enumerate(WAVE1):
        TAGOF[g] = f"b{i}"
    for i, g in enumerate(WAVE2):
        TAGOF[g] = f"b{i}"
    TAGOF[WAVE3[0]] = "b0"
    TAGOF[WAVE3[1]] = "b1"

    M = {}

    def run_wave(wave, wname, sig_slots=False):
        for g in wave:
            M[g] = psum.tile([128, H, NTX], F32, tag=TAGOF[g], name=f"{wname}{g}")
        for c in range(NCH):
            for dy in range(KK):
                if sig_slots and c == NCH - 2 and dy > 0:
                    sig_col(2 * (dy - 1))
                    sig_col(2 * (dy - 1) + 1)
                first = c == 0 and dy == 0
                last = c == NCH - 1 and dy == KK - 1
                for p, oc in wave:
                    nc.tensor.matmul(
                        M[(p, oc)][:],
                        lhsT(p, c, dy, oc),
                        T[c][p][:, dy : dy + H, :],
                        start=first,
                        stop=last,
                    )

    run_wave(WAVE1, "w1")

    # boundary 1: park M0 (all oc) and M3 (oc0..2) in SBUF via ACT copies
    # (b0's copy first so wave 2 can reclaim its bank immediately)
    y0p, y1n = {}, {}
    for oc in range(NCH):
        y0p[oc] = ypool.tile([128, H, NTX], F32, tag=f"y0p{oc}", name=f"y0p{oc}")
        nc.scalar.activation(y0p[oc][:], M[(0, oc)][:], AF.Copy)
    for oc in range(3):
        y1n[oc] = ypool.tile([128, H, NTX], F32, tag=f"y1n{oc}", name=f"y1n{oc}")
        nc.scalar.activation(y1n[oc][:], M[(3, oc)][:], AF.Copy)

    run_wave(WAVE2, "w2", sig_slots=True)

    # sigma_inv = 1/sqrt(RC^2*q + eps) -- right after wave 2 so the oc0..2
    # flushes overlap wave 3
    nc.vector.tensor_scalar(
        sig_f[:], sig_q[:, 0 : 2 * NCH : 2], RC * RC, EPS, ALU.mult, ALU.add
    )
    nc.scalar.activation(sig_s[:], sig_f[:], AF.Sqrt)
    nc.vector.reciprocal(sig_t[:], sig_s[:])

    def flush(oc, Y0, Y1):
        ob = opool.tile([128, H, W], F32, tag="ob", name="ob")
        obv = ob[:].rearrange("p y (t k) -> p y t k", k=2)
        sg = sig_t[:, oc : oc + 1]
        nc.scalar.activation(obv[:, :, :, 0], Y0, AF.Copy, scale=sg)
        nc.scalar.activation(obv[:, :, :, 1], Y1, AF.Copy, scale=sg)
        nc.sync.dma_start(out_d[oc], ob[:].rearrange("p y w -> p (y w)"))

    # wave2-end: finals for oc0..2: Y0 = (M1+M2)+M0s, Y1 = (M1-M2)-M3s;
    # park M3[oc3]. The sigma matmuls ride wave 3's dy==1 slots, and the
    # sigma finalize chain hides under wave 3 / the first flushes.
    y1n3 = ypool.tile([128, H, NTX], F32, tag="y1n3")
    nc.scalar.activation(y1n3[:], M[(3, 3)][:], AF.Copy)
    y0f, y1f = {}, {}
    for oc in range(3):
        y0f[oc] = ypool.tile([128, H, NTX], F32, tag=f"y0f{oc}", name=f"y0f{oc}")
        y1f[oc] = ypool.tile([128, H, NTX], F32, tag=f"y1f{oc}", name=f"y1f{oc}")
        # one PSUM operand per DVE op: associate through the SBUF partials
        nc.vector.tensor_add(y0f[oc][:], M[(1, oc)][:], y0p[oc][:])
        nc.vector.tensor_add(y0f[oc][:], y0f[oc][:], M[(2, oc)][:])
        nc.vector.tensor_sub(y1f[oc][:], M[(1, oc)][:], y1n[oc][:])
        nc.vector.tensor_sub(y1f[oc][:], y1f[oc][:], M[(2, oc)][:])

    # wave 3: run (1,3)'s 12 matmuls to completion first, then (2,3)'s, so
    # the oc3 final chain can start while (2,3) is still streaming.
    g13, g23 = WAVE3
    M[g13] = psum.tile([128, H, NTX], F32, tag=TAGOF[g13], name="w3a")
    M[g23] = psum.tile([128, H, NTX], F32, tag=TAGOF[g23], name="w3b")
    for g in (g13, g23):
        k = 0
        for c in range(NCH):
            for dy in range(KK):
                p, oc = g
                nc.tensor.matmul(
                    M[g][:],
                    lhsT(p, c, dy, oc),
                    T[c][p][:, dy : dy + H, :],
                    start=k == 0,
                    stop=k == NCH * KK - 1,
                )
                k += 1

    for oc in range(3):
        flush(oc, y0f[oc][:], y1f[oc][:])

    y0f3 = ypool.tile([128, H, NTX], F32, tag="y0f3")
    y1f3 = ypool.tile([128, H, NTX], F32, tag="y1f3")
    nc.vector.tensor_add(y0f3[:], M[(1, 3)][:], y0p[3][:])
    nc.vector.tensor_sub(y1f3[:], M[(1, 3)][:], y1n3[:])
    nc.vector.tensor_add(y0f3[:], y0f3[:], M[(2, 3)][:])
    nc.vector.tensor_sub(y1f3[:], y1f3[:], M[(2, 3)][:])
    flush(3, y0f3[:], y1f3[:])


_CACHE = None


def _get_compiled():
    global _CACHE
    if _CACHE is None:
        nc = bacc.Bacc("TRN2", target_bir_lowering=False, debug=False, num_devices=B)
        x_d = nc.dram_tensor("x", [NCH, 128, PIX], BF16, kind="ExternalInput").ap()
        st_d = nc.dram_tensor("style", [128, NCH], F32, kind="ExternalInput").ap()
        wt_d = nc.dram_tensor(
            "wt", [128, NCH, TAPS, COUT], BF16, kind="ExternalInput"
        ).ap()
        out_d = nc.dram_tensor("out", [NCH, 128, PIX], F32, kind="ExternalOutput").ap()
        with tile.TileContext(nc) as tc, ExitStack() as ctx:
            _body(ctx, tc, x_d, st_d, wt_d, out_d)
        nc.compile()
        _CACHE = nc
    return _CACHE


def _trunc_bf16(a):
    hi = a.view(np.uint16).reshape(*a.shape, 2)[..., 1]
    return np.ascontiguousarray(hi).view(ml_dtypes.bfloat16)


def kernel(x, style, weight):
    global LAST_RESULTS
    x = np.ascontiguousarray(np.asarray(x, dtype=np.float32))
    style = np.asarray(style, dtype=np.float32)
    weight = np.ascontiguousarray(np.asarray(weight, dtype=np.float32))

    wt_hi = weight.view(np.uint16).reshape(COUT, NCH, 128, TAPS, 2)[..., 1]
    wt = np.ascontiguousarray(wt_hi.transpose(2, 1, 3, 0)).view(ml_dtypes.bfloat16)
    in_maps = []
    for b in range(B):
        in_maps.append(
            {
                "x": _trunc_bf16(x[b]).reshape(NCH, 128, PIX),
                "style": np.ascontiguousarray(style[b].reshape(NCH, 128).T),
                "wt": wt,
            }
        )

    nc = _get_compiled()
    res = run_bass_kernel_spmd(nc, in_maps, list(range(B)), trace=TRACE)
    LAST_RESULTS = res
    out = np.empty((B, COUT, H, W), dtype=np.float32)
    for b in range(B):
        out[b] = res.results[b]["out"].reshape(COUT, H, W)
    return out
